# revision 16
# baseline (speedup 1.0000x reference)
"""Trainium2 Bass kernel for nn_BanditPrototypeManager.

Data-parallel across (B,N): 16 object pairs sharded 2-per-core over 8 cores.
Only `conditioned` is returned by the reference; MLP/context/logits and
age/usage/conf updates are dead code.  Live dataflow per (b,n):
  cand  = l2norm(masked-pool(value, mask))
  sim   = bank_n @ cand ; action/slot rule      (tiny)
  proto_new, valid_new  (EMA scatter, 1 slot)   (tiny)
  rsim  = l2norm(proto_new) @ (value/||value||_C)
  attn  = softmax_K(rsim masked by valid_new)
  out   = value + pg * attn^T proto_new + fg * frame_feat

Cost-model-aware layout (v2):
  - value resident [C=2x128 part, HW free] for BOTH pairs ([128,4608] x4/pair)
  - frame pre-cast to bf16 on host, resident (36KB/part total)
  - stage A: PE transposes (f32r, 1.5cyc/row) -> vt_sb; nsq via DVE STT
    (fp32 SBUF 2x mode); cand via 144 out-free-1 PE matmuls (lhsT=vt chunk,
    rhs=mhat column) accumulating into psum [128c, 2]
  - stage B: bank/slot logic, norms via DVE tensor_scalar(max,pow) rsqrt
    (no Act Sqrt -> no act-table thrash)
  - stage C1: rsim s_T[hw,8] PE matmuls + pen row; rlog = s*rinv via one
    broadcast-AP DVE mult per psum tile
  - stage C2: one exp (Act), Z-reduce + reciprocal, anorm(bf16) via one
    broadcast-AP DVE mult
  - stage C3: bf16 attn transposes -> attn_sb; psum_o = P2r@attn (f32r/bf16
    1cyc) + fgI@frame(bf16) inject; final add either DVE (psum+value in
    place into val_sb) or PE value-inject + Act copy; out DMA from val_sb
  - DMA queues: SP = value+frame+smalls, Pool(SWDGE) = out stores; the cost
    model overlaps transfers across queues.
"""

import os
import sys

if "/opt/trn_rl_repo" not in sys.path:
    sys.path.insert(0, "/opt/trn_rl_repo")

import numpy as np

B, N, K, C, H, W = 2, 8, 8, 256, 96, 96
HW = H * W  # 9216
ALPHA = 0.3
SIM_HIGH, SIM_LOW = 0.8, 0.3
NCORES = 8
PAIRS = 2
GW = 4608            # value tile width ([128, GW] tiles, 2 per cb)
NG = HW // GW        # 2 tile groups
NJ = HW // 128       # 72 transpose chunks
JPG = GW // 128      # 36 chunks per tile group
NQ = GW // 512       # 9 out chunks per (cb, G) tile
PEN = -1e9

_nc_cache = None


def build_nc():
    import concourse.bass as bass
    import concourse.bacc as bacc
    import concourse.mybir as mybir
    import concourse.tile as tile
    from concourse.masks import make_identity

    fp32 = mybir.dt.float32
    f32r = mybir.dt.float32r
    bf16 = mybir.dt.bfloat16
    Alu = mybir.AluOpType
    Act = mybir.ActivationFunctionType

    nc = bacc.Bacc()

    value_d = nc.declare_dram_parameter("value", [PAIRS, C, HW], fp32, isOutput=False)
    frame_d = nc.declare_dram_parameter("frameb", [C, HW], bf16, isOutput=False)
    mhat_d = nc.declare_dram_parameter("mhat", [PAIRS, HW], fp32, isOutput=False)
    bank_d = nc.declare_dram_parameter("bank", [PAIRS, K, C], fp32, isOutput=False)
    proto_d = nc.declare_dram_parameter("protot", [PAIRS, K, C], fp32, isOutput=False)
    valid_d = nc.declare_dram_parameter("validf", [PAIRS, 1, K], fp32, isOutput=False)
    spawn_d = nc.declare_dram_parameter("spawn", [PAIRS, 1, K], fp32, isOutput=False)
    pg_d = nc.declare_dram_parameter("pg8", [K, 1], fp32, isOutput=False)
    fg_d = nc.declare_dram_parameter("fg128", [128, 1], fp32, isOutput=False)
    out_d = nc.declare_dram_parameter("out", [PAIRS, C, HW], fp32, isOutput=True)

    from contextlib import ExitStack

    with tile.TileContext(nc) as tc, ExitStack() as ctx:
        # ---------------- pools ----------------
        pval = ctx.enter_context(tc.tile_pool(name="pval", bufs=7))
        pframe = ctx.enter_context(tc.tile_pool(name="pframe", bufs=1))
        pconst = ctx.enter_context(tc.tile_pool(name="pconst", bufs=1))
        pbig = ctx.enter_context(tc.tile_pool(name="pbig", bufs=1))
        pbig2 = ctx.enter_context(tc.tile_pool(name="pbig2", bufs=2))
        psmallsb = ctx.enter_context(tc.tile_pool(name="psmallsb", bufs=2))
        pctl = ctx.enter_context(tc.tile_pool(name="pctl", bufs=2))
        pctl1 = ctx.enter_context(tc.tile_pool(name="pctl1", bufs=1))
        pvts = ctx.enter_context(tc.tile_pool(name="pvts", bufs=2))
        pscr = ctx.enter_context(tc.tile_pool(name="pscr", bufs=1))
        patts = ctx.enter_context(tc.tile_pool(name="patts", bufs=4))

        ps_vt = ctx.enter_context(tc.tile_pool(name="ps_vt", bufs=2, space="PSUM"))
        ps_cand = ctx.enter_context(tc.tile_pool(name="ps_cand", bufs=1, space="PSUM"))
        ps_small = ctx.enter_context(tc.tile_pool(name="ps_small", bufs=1, space="PSUM"))
        ps_st = ctx.enter_context(tc.tile_pool(name="ps_st", bufs=1, space="PSUM"))
        ps_at = ctx.enter_context(tc.tile_pool(name="ps_at", bufs=1, space="PSUM"))
        ps_out = ctx.enter_context(tc.tile_pool(name="ps_out", bufs=2, space="PSUM"))

        # ---------------- constants ----------------
        ident = pconst.tile([128, 128], fp32, name="ident")
        make_identity(nc, ident[:])
        fg128 = pconst.tile([128, 1], fp32, name="fg128sb")
        nc.sync.dma_start(fg128[:], fg_d[:])
        identb = pconst.tile([128, 128], bf16, name="identb")
        nc.scalar.copy(identb[:], ident[:])
        fgIb = pconst.tile([128, 128], bf16, name="fgIb")
        nc.vector.tensor_scalar_mul(fgIb[:], ident[:], fg128[:, 0:1])
        ones_1x8 = pconst.tile([1, 8], fp32, name="ones_1x8")
        nc.gpsimd.memset(ones_1x8[:], 1.0)
        ones_1x128 = pconst.tile([1, 128], fp32, name="ones_1x128")
        nc.gpsimd.memset(ones_1x128[:], 1.0)
        iota_i = pconst.tile([1, 8], mybir.dt.int32, name="iota_i")
        nc.gpsimd.iota(iota_i[:], pattern=[[1, 8]], base=0, channel_multiplier=0)
        iota_f = pconst.tile([1, 8], fp32, name="iota_f")
        nc.vector.tensor_copy(iota_f[:], iota_i[:])
        pg8 = pconst.tile([K, 1], fp32, name="pg8sb")
        nc.sync.dma_start(pg8[:], pg_d[:])

        # frame resident bf16 (loaded once, interleaved into SP queue below)
        frame_sb = [pframe.tile([128, HW], bf16, name=f"frame_{cb}") for cb in range(2)]

        for p in range(PAIRS):
            # ---------------- value loads (SP queue) ----------------
            val_sb = [[None] * NG for _ in range(2)]
            for G in range(NG):
                for cb in range(2):
                    vt = pval.tile([128, GW], fp32, name="valt", tag="valt")
                    nc.sync.dma_start(
                        vt[:],
                        value_d[p, cb * 128:(cb + 1) * 128, G * GW:(G + 1) * GW],
                    )
                    val_sb[cb][G] = vt
                if p == 0 and G == 0:
                    # frame cb0 after pair0's first tile group
                    nc.sync.dma_start(frame_sb[0][:], frame_d[0:128, :])
                if p == 0 and G == 1:
                    nc.sync.dma_start(frame_sb[1][:], frame_d[128:256, :])
            mhat_sb = psmallsb.tile([128, NJ], fp32, name="mhat_sb")
            nc.sync.dma_start(mhat_sb[:], mhat_d[p].rearrange("(j q) -> q j", q=128))
            bank_sb = pctl.tile([K, C], fp32, name="bank_sb")
            nc.sync.dma_start(bank_sb[:], bank_d[p])
            proto_sb = pctl.tile([K, C], fp32, name="proto_sb")
            nc.sync.dma_start(proto_sb[:], proto_d[p])
            validT = pctl.tile([1, K], fp32, name="validT")
            nc.sync.dma_start(validT[:], valid_d[p])
            spawnT = pctl.tile([1, K], fp32, name="spawnT")
            nc.sync.dma_start(spawnT[:], spawn_d[p])

            nsq = psmallsb.tile([128, NJ], fp32, name="nsq")
            psum_cand = ps_cand.tile([128, 2], fp32, name="psum_cand")

            # ---------------- stage A: transpose, nsq, cand ----------------
            for jj in range(NJ // 2):
                psum_vt = ps_vt.tile([128, 512], fp32, name="psum_vt")
                for u in range(2):
                    j = 2 * jj + u
                    G, off = j // JPG, (j % JPG) * 128
                    for cb in range(2):
                        nc.tensor.transpose(
                            psum_vt[:, 256 * u + 128 * cb:256 * u + 128 * cb + 128],
                            val_sb[cb][G][:, off:off + 128],
                            ident[:],
                        )
                vt_sb = pvts.tile([128, 512], fp32, name="vt_sb")
                if jj % 4 == 3:
                    nc.vector.tensor_copy(vt_sb[:], psum_vt[:])
                else:
                    nc.scalar.copy(vt_sb[:], psum_vt[:])
                scr = pscr.tile([128, 512], fp32, name="scr")
                for u in range(2):
                    j = 2 * jj + u
                    sl = vt_sb[:, 256 * u:256 * u + 256]
                    nc.vector.scalar_tensor_tensor(
                        scr[:, 256 * u:256 * u + 256],
                        in0=sl, scalar=1.0, in1=sl,
                        op0=Alu.mult, op1=Alu.mult,
                        accum_out=nsq[:, j:j + 1],
                    )
                    # cand accumulation: psum_cand[:, h] += vt_chunk^T @ mhat_col
                    for h in range(2):
                        nc.tensor.matmul(
                            psum_cand[:, h:h + 1],
                            lhsT=vt_sb[:, 256 * u + 128 * h:256 * u + 128 * h + 128],
                            rhs=mhat_sb[:, j:j + 1],
                            start=(j == 0 and h == 0),
                            stop=(j == NJ - 1 and h == 1),
                        )

            if os.environ.get("KSTAGE") == "A":
                continue
            # rinv per pixel: (max(nsq,1e-24))^-0.5  == 1/max(sqrt(nsq),1e-12)
            nsr = psmallsb.tile([128, NJ], fp32, name="nsr")
            nc.scalar.activation(nsr[:], nsq[:], Act.Sqrt)
            nc.vector.tensor_scalar_max(nsr[:], nsr[:], 1e-12)
            rinv = psmallsb.tile([128, NJ], fp32, name="rinv")
            nc.vector.reciprocal(rinv[:], nsr[:])

            # ---------------- stage B: bank / slot logic ----------------
            cand2 = pctl1.tile([128, 2], fp32, name="cand2")
            nc.scalar.copy(cand2[:], psum_cand[:])
            psum_c2T = ps_small.tile([2, 128], fp32, name="psum_c2T", tag="psmall")
            nc.tensor.transpose(psum_c2T[:], cand2[:], ident[:])
            c2sb = pctl.tile([2, 128], fp32, name="c2sb")
            nc.scalar.copy(c2sb[:], psum_c2T[:])
            cand_row = pctl.tile([1, C], fp32, name="cand_row")
            nc.vector.tensor_copy(cand_row[:, 0:128], c2sb[0:1, :])
            # partition-1 row move needs DMA (engines can't start at partition 1);
            # Pool queue is idle here and keeps SP free for value loads
            nc.gpsimd.dma_start(cand_row[:, 128:256], c2sb[1:2, :])
            scr1 = pctl1.tile([1, C], fp32, name="scr1")
            cnsq = pctl.tile([1, 1], fp32, name="cnsq")
            nc.vector.scalar_tensor_tensor(
                scr1[:], in0=cand_row[:], scalar=1.0, in1=cand_row[:],
                op0=Alu.mult, op1=Alu.mult, accum_out=cnsq[:],
            )
            cnrm = pctl.tile([1, 1], fp32, name="cnrm")
            nc.scalar.activation(cnrm[:], cnsq[:], Act.Sqrt)
            nc.vector.tensor_scalar_max(cnrm[:], cnrm[:], 1e-12)
            crinv = pctl.tile([1, 1], fp32, name="crinv")
            nc.vector.reciprocal(crinv[:], cnrm[:])
            cand_n = pctl.tile([1, C], fp32, name="cand_n")
            nc.vector.tensor_scalar_mul(cand_n[:], cand_row[:], crinv[:])

            psum_c8 = ps_small.tile([K, C], fp32, name="psum_c8", tag="psmall")
            nc.tensor.matmul(psum_c8[:], lhsT=ones_1x8[:], rhs=cand_n[:],
                             start=True, stop=True)
            cand_b8 = pctl1.tile([K, C], fp32, name="cand_b8")
            nc.scalar.copy(cand_b8[:], psum_c8[:])

            scr8 = pctl1.tile([K, C], fp32, name="scr8")
            sim8 = pctl.tile([K, 1], fp32, name="sim8")
            nc.vector.scalar_tensor_tensor(
                scr8[:], in0=bank_sb[:], scalar=1.0, in1=cand_b8[:],
                op0=Alu.mult, op1=Alu.mult, accum_out=sim8[:],
            )
            psum_s1 = ps_small.tile([1, K], fp32, name="psum_s1", tag="psmall")
            nc.tensor.transpose(psum_s1[:], sim8[:], ident[:8, :8])
            simT = pctl.tile([1, K], fp32, name="simT")
            nc.scalar.copy(simT[:], psum_s1[:])

            # sim masked: valid ? sim : -1
            t1 = pctl.tile([1, K], fp32, name="t1")
            nc.vector.tensor_mul(t1[:], simT[:], validT[:])
            t2 = pctl.tile([1, K], fp32, name="t2")
            nc.vector.tensor_scalar_add(t2[:], validT[:], -1.0)
            sim_m = pctl.tile([1, K], fp32, name="sim_m")
            nc.vector.tensor_add(sim_m[:], t1[:], t2[:])

            mx8 = pctl.tile([1, 8], fp32, name="mx8")
            nc.vector.max(mx8[:], sim_m[:])
            mi8 = pctl.tile([1, 8], mybir.dt.uint32, name="mi8")
            nc.vector.max_index(mi8[:], mx8[:], sim_m[:])
            tgt_f = pctl.tile([1, 1], fp32, name="tgt_f")
            nc.vector.tensor_copy(tgt_f[:], mi8[:, 0:1])

            any_v = pctl.tile([1, 1], fp32, name="any_v")
            nc.vector.tensor_reduce(any_v[:], validT[:], axis=mybir.AxisListType.X,
                                    op=Alu.max)
            hi = pctl.tile([1, 1], fp32, name="hi")
            nc.vector.tensor_scalar(hi[:], mx8[:, 0:1], SIM_HIGH, None, Alu.is_ge)
            lo = pctl.tile([1, 1], fp32, name="lo")
            nc.vector.tensor_scalar(lo[:], mx8[:, 0:1], SIM_LOW, None, Alu.is_ge)
            rf = pctl.tile([1, 1], fp32, name="rf")
            nc.vector.tensor_mul(rf[:], any_v[:], hi[:])
            avlo = pctl.tile([1, 1], fp32, name="avlo")
            nc.vector.tensor_mul(avlo[:], any_v[:], lo[:])
            wf = pctl.tile([1, 1], fp32, name="wf")
            nc.vector.tensor_scalar(wf[:], avlo[:], -1.0, 1.0, Alu.mult, Alu.add)

            oh_t = pctl.tile([1, K], fp32, name="oh_t")
            nc.vector.tensor_scalar(oh_t[:], iota_f[:], tgt_f[:], None, Alu.is_equal)
            rmT = pctl.tile([1, K], fp32, name="rmT")
            nc.vector.tensor_scalar_mul(rmT[:], oh_t[:], rf[:])
            wmT = pctl.tile([1, K], fp32, name="wmT")
            nc.vector.tensor_scalar_mul(wmT[:], spawnT[:], wf[:])
            vnewT = pctl.tile([1, K], fp32, name="vnewT")
            nc.vector.tensor_max(vnewT[:], validT[:], wmT[:])

            psum_rw = ps_small.tile([K, 2], fp32, name="psum_rw", tag="psmall")
            nc.tensor.transpose(psum_rw[:, 0:1], rmT[:], ident[:1, :1])
            nc.tensor.transpose(psum_rw[:, 1:2], wmT[:], ident[:1, :1])
            rw8 = pctl.tile([K, 2], fp32, name="rw8")
            nc.scalar.copy(rw8[:], psum_rw[:])

            # refined = l2norm(0.7*proto + 0.3*cand)
            c3 = pctl1.tile([K, C], fp32, name="c3")
            nc.vector.tensor_scalar_mul(c3[:], cand_b8[:], ALPHA)
            pre = pctl1.tile([K, C], fp32, name="pre")
            nc.vector.scalar_tensor_tensor(
                pre[:], in0=proto_sb[:], scalar=1.0 - ALPHA, in1=c3[:],
                op0=Alu.mult, op1=Alu.add,
            )
            rn = pctl.tile([K, 1], fp32, name="rn")
            nc.vector.scalar_tensor_tensor(
                scr8[:], in0=pre[:], scalar=1.0, in1=pre[:],
                op0=Alu.mult, op1=Alu.mult, accum_out=rn[:],
            )
            nc.scalar.activation(rn[:], rn[:], Act.Sqrt)
            nc.vector.tensor_scalar_max(rn[:], rn[:], 1e-12)
            rri = pctl.tile([K, 1], fp32, name="rri")
            nc.vector.reciprocal(rri[:], rn[:])
            refined = pctl1.tile([K, C], fp32, name="refined")
            nc.vector.tensor_scalar_mul(refined[:], pre[:], rri[:])

            # proto_new = proto + rm*(refined-proto) + wm*(cand-proto)
            d1 = pctl1.tile([K, C], fp32, name="d1")
            nc.vector.tensor_sub(d1[:], refined[:], proto_sb[:])
            p1 = pctl1.tile([K, C], fp32, name="p1")
            nc.vector.scalar_tensor_tensor(
                p1[:], in0=d1[:], scalar=rw8[:, 0:1], in1=proto_sb[:],
                op0=Alu.mult, op1=Alu.add,
            )
            d2 = pctl1.tile([K, C], fp32, name="d2")
            nc.vector.tensor_sub(d2[:], cand_b8[:], proto_sb[:])
            pnew = pctl.tile([K, C], fp32, name="pnew")
            nc.vector.scalar_tensor_tensor(
                pnew[:], in0=d2[:], scalar=rw8[:, 1:2], in1=p1[:],
                op0=Alu.mult, op1=Alu.add,
            )

            # pnn = l2norm(proto_new); P2 = pg * proto_new
            nn2 = pctl.tile([K, 1], fp32, name="nn2")
            nc.vector.scalar_tensor_tensor(
                scr8[:], in0=pnew[:], scalar=1.0, in1=pnew[:],
                op0=Alu.mult, op1=Alu.mult, accum_out=nn2[:],
            )
            nc.scalar.activation(nn2[:], nn2[:], Act.Sqrt)
            nc.vector.tensor_scalar_max(nn2[:], nn2[:], 1e-12)
            nri = pctl.tile([K, 1], fp32, name="nri")
            nc.vector.reciprocal(nri[:], nn2[:])
            pnn = pctl.tile([K, C], fp32, name="pnn")
            nc.vector.tensor_scalar_mul(pnn[:], pnew[:], nri[:])
            P2 = pctl.tile([K, C], bf16, name="P2")
            nc.vector.tensor_scalar_mul(P2[:], pnew[:], pg8[:])
            # penT last among stage-B DVE products: C1's wait on it covers all
            penT = pctl.tile([1, K], fp32, name="penT")
            nc.vector.tensor_scalar(penT[:], vnewT[:], -1.0, -PEN, Alu.add, Alu.mult)

            psum_pc = ps_small.tile([128, 16], fp32, name="psum_pc", tag="psmall")
            nc.tensor.transpose(psum_pc[:, 0:8], pnn[:, 0:128], ident[:8, :8])
            nc.tensor.transpose(psum_pc[:, 8:16], pnn[:, 128:256], ident[:8, :8])
            pnnc = pctl.tile([128, 16], fp32, name="pnnc")
            nc.scalar.copy(pnnc[:], psum_pc[:])

            if os.environ.get("KSTAGE") == "B":
                continue
            # ---------------- stage C1: s_T + rlog ----------------
            rlog = pbig.tile([128, NJ * 8], fp32, name="rlog")
            for i in range(NJ // 4):
                psum_st = ps_st.tile([128, 32], fp32, name="psum_st")
                for u in range(4):
                    j = 4 * i + u
                    G, off = j // JPG, (j % JPG) * 128
                    sl = psum_st[:, 8 * u:8 * u + 8]
                    nc.tensor.matmul(sl, lhsT=val_sb[0][G][:, off:off + 128],
                                     rhs=pnnc[:, 0:8], start=True, stop=False)
                    nc.tensor.matmul(sl, lhsT=val_sb[1][G][:, off:off + 128],
                                     rhs=pnnc[:, 8:16], start=False, stop=False)
                    nc.tensor.matmul(sl, lhsT=ones_1x128[:], rhs=penT[:],
                                     start=False, stop=True)
                nc.vector.tensor_tensor(
                    rlog[:, 32 * i:32 * i + 32].rearrange("p (j k) -> p j k", k=8),
                    psum_st[:].rearrange("p (j k) -> p j k", k=8),
                    rinv[:, 4 * i:4 * i + 4].broadcast_to([128, 4, 8]),
                    op=Alu.mult,
                )

            if os.environ.get("KSTAGE") == "C1":
                continue
            # ---------------- stage C2: softmax over K ----------------
            e = rlog
            nc.scalar.activation(e[:], rlog[:], Act.Exp)
            Zt = psmallsb.tile([128, NJ], fp32, name="Zt")
            nc.vector.tensor_reduce(
                Zt[:], e[:].rearrange("p (j k) -> p j k", k=8),
                axis=mybir.AxisListType.X, op=Alu.add,
            )
            rz = psmallsb.tile([128, NJ], fp32, name="rz")
            nc.vector.reciprocal(rz[:], Zt[:])
            anorm = pbig2.tile([128, NJ * 8], bf16, name="anorm")
            nc.vector.tensor_tensor(
                anorm[:].rearrange("p (j k) -> p j k", k=8),
                e[:].rearrange("p (j k) -> p j k", k=8),
                rz[:].broadcast_to([128, NJ, 8]),
                op=Alu.mult,
            )

            if os.environ.get("KSTAGE") == "C2":
                continue
            # ---------------- stage C3: attn, pmap, out ----------------
            for G in range(NG):
                for q in range(NQ):
                    i = G * JPG // 4 + q
                    qoff = q * 512
                    hw0 = G * GW + qoff
                    psum_at = ps_at.tile([8, 512], bf16, name="psum_at")
                    for u in range(4):
                        j = 4 * i + u
                        nc.tensor.transpose(
                            psum_at[:, 128 * u:128 * u + 128],
                            anorm[:, 8 * j:8 * j + 8],
                            identb[:],
                        )
                    attn_sb = patts.tile([8, 512], bf16, name="attn_sb")
                    nc.scalar.copy(attn_sb[:], psum_at[:])
                    for cb in range(2):
                        psum_o = ps_out.tile([128, 512], fp32, name="psum_o")
                        nc.tensor.matmul(
                            psum_o[:],
                            lhsT=P2[:, 128 * cb:128 * cb + 128],
                            rhs=attn_sb[:], start=True, stop=False)
                        nc.tensor.matmul(
                            psum_o[:], lhsT=fgIb[:],
                            rhs=frame_sb[cb][:, hw0:hw0 + 512],
                            start=False, stop=True)
                        vsl = val_sb[cb][G][:, qoff:qoff + 512]
                        nc.vector.tensor_add(vsl, psum_o[:], vsl)
                for cb in range(2):
                    # out store for this (cb, G) tile on the Pool queue
                    nc.gpsimd.dma_start(
                        out_d[p, cb * 128:(cb + 1) * 128, G * GW:(G + 1) * GW],
                        val_sb[cb][G][:],
                    )

    nc.compile()
    return nc


def get_nc():
    global _nc_cache
    if _nc_cache is None:
        _nc_cache = build_nc()
    return _nc_cache


def host_prep(value, frame_feat, mask, proto, age, usage, conf,
              proto_gate, frame_gate, valid):
    """Input-only host preprocessing (all tiny except reshapes/casts)."""
    import ml_dtypes
    fv = np.asarray(value, np.float32).reshape(B, N, C, HW)
    m = np.asarray(mask, np.float32).reshape(B, N, HW)
    msum = m.sum(-1)
    denom = np.maximum(msum, np.float32(1e-6))
    mhat = np.where((denom <= 1e-5)[..., None], np.float32(1.0 / HW),
                    m / denom[..., None]).astype(np.float32)

    proto = np.asarray(proto, np.float32)
    nrm = np.maximum(np.sqrt((proto * proto).sum(-1, keepdims=True)),
                     np.float32(1e-12))
    bank = (proto / nrm).astype(np.float32)

    age = np.asarray(age, np.float32)
    usage = np.asarray(usage, np.float32)
    conf = np.asarray(conf, np.float32)
    valid = np.asarray(valid, bool)
    age_n = age / max(float(age.max()), 1.0)
    usage_n = usage / max(float(usage.max()), 1.0)
    victim = np.argmax(age_n + (1.0 - usage_n) + (1.0 - conf), axis=-1)
    has_empty = (~valid).any(-1)
    first_empty = np.argmax(~valid, axis=-1)
    spawn = np.where(has_empty, first_empty, victim)
    spawn_oh = np.zeros((B, N, K), np.float32)
    bb, nn_ = np.meshgrid(np.arange(B), np.arange(N), indexing="ij")
    spawn_oh[bb, nn_, spawn] = 1.0
    validf = valid.astype(np.float32)
    frameb = np.asarray(frame_feat, np.float32).reshape(B, C, HW).astype(
        ml_dtypes.bfloat16)
    return fv, frameb, mhat, bank, proto, validf, spawn_oh


def make_in_maps(value, frame_feat, mask, proto, age, usage, conf,
                 proto_gate, frame_gate, valid):
    fv, frameb, mhat, bank, proto, validf, spawn_oh = host_prep(
        value, frame_feat, mask, proto, age, usage, conf,
        proto_gate, frame_gate, valid)
    pg8 = np.full((K, 1), np.float32(proto_gate), np.float32)
    fg128 = np.full((128, 1), np.float32(frame_gate), np.float32)
    in_maps = []
    for c in range(NCORES):
        b, n0 = c // 4, 2 * (c % 4)
        in_maps.append(dict(
            value=np.ascontiguousarray(fv[b, n0:n0 + 2]),
            frameb=np.ascontiguousarray(frameb[b]),
            mhat=np.ascontiguousarray(mhat[b, n0:n0 + 2]),
            bank=np.ascontiguousarray(bank[b, n0:n0 + 2]),
            protot=np.ascontiguousarray(proto[b, n0:n0 + 2]),
            validf=np.ascontiguousarray(validf[b, n0:n0 + 2].reshape(PAIRS, 1, K)),
            spawn=np.ascontiguousarray(spawn_oh[b, n0:n0 + 2].reshape(PAIRS, 1, K)),
            pg8=pg8, fg128=fg128,
        ))
    return in_maps


def kernel(value, frame_feat, mask, proto, age, usage, conf,
           W1, b1, W2, b2, proto_gate, frame_gate, valid,
           _results_hook=None):
    from concourse.bass_utils import run_bass_kernel_spmd

    nc = get_nc()
    in_maps = make_in_maps(value, frame_feat, mask, proto, age, usage, conf,
                           proto_gate, frame_gate, valid)
    res = run_bass_kernel_spmd(nc, in_maps, core_ids=list(range(NCORES)))
    if _results_hook is not None:
        _results_hook(res)
    out = np.empty((B, N, C, H, W), np.float32)
    for c in range(NCORES):
        b, n0 = c // 4, 2 * (c % 4)
        out[b, n0:n0 + 2] = res.results[c]["out"].reshape(PAIRS, C, H, W)
    return out
